# revision 38
# baseline (speedup 1.0000x reference)
"""Trainium2 Bass kernel for nn_Classifier_69818988363910 (segment_reduce).

Reference computation (after dead-code elimination):
    local = relu(x @ W1^T)                        # [60000, 2048]
    feats = local.reshape(2000, 30, 2048).mean(1) # [2000, 2048]
    logits = concat(feats, feats) @ Wlin^T        # [2000, 1000]
           = feats @ (Wlin[:, :2048] + Wlin[:, 2048:])^T
y / W2 are computed but unused in the reference (original-code bug), so the
output depends only on x, W1, Wlin.

Sharding: data-parallel over the 8 NeuronCores along T (7500 rows = 250
segments per core); W1 / Wc replicated. No collectives; host gathers.

Device kernel per core (fp32 accumulation in PSUM throughout):
    MM1 on PE:   z[e, t] = sum_d W1T[d, e] * xT[d, t]
                 bf16 mode: 8 k-tiles of 128;  fp8 mode: 4 DoubleRow
                 super-k-tiles of 256 (2x PE rate)
    relu on ACT: psum -> sbuf
    pool on DVE: tensor_reduce over [128, segs, 30] view (sum; the 1/30
                 mean scale and the fp8 W1 pre-scale are folded into Wc
                 on the host)
    MM2 on PE:   logits[s, c] = sum_e featsT[e, s] * WcT[e, c]  (bf16)

Perf notes (trace-driven; staged baseline 237.2us -> ~235.1-236.0us):
  - The PE runs MM1 at the fp8-DoubleRow peak (202ns per 256x128x480
    matmul, 0.42ns per moving row); MM1 202.7us + MM2 13.4us of PE
    stream is the hard floor.  This version runs the stream with
    <0.7us of total stalls; the rest is startup + writeback tail.
  - Framework preamble ends ~6.9us; DMA transfers can only start
    ~8.5us (issue + ring bring-up), and each DMA's completion
    semaphore costs ~900ns after the data lands.  Only sync (SP) and
    scalar (ACT) have HWDGE rings; total fabric ~390 B/ns, one ring
    can pull ~195 B/ns while the other is active.
  - Startup schedule (first real matmul gates on kt0-h0 + x0-kt0pair,
    one ~125-256KB piece per HWDGE queue):
      scalar: micro, W1 h0 half-slabs kt0, kt1 (prepacked
              [kt][half][p][j][e'] -> one 2KB descriptor/partition).
      sync:   micro, x0(d0:2), x0(d2:4), kt2-h0, x0(d4:8), kt3-h0,
              x1, then the h1 half-slabs (needed only from e8,
              ~7us after open), then x chunks 2..15 with the 4MB wct
              DMA behind chunk-4's x (pool-slot gating pushes its
              transfer past the startup window; needed ~215us in).
    Tile builds the dependency graph from EMISSION order, so chunk-1's
    x and the h1 slabs are emitted before the chunk-0 matmul bodies
    that must wait on them.
  - Chunk 0 compute: kt-outer wave over e0..6 (each 1.4us kt row
    covers the next half-slab's delivery), then kt-inner per e from
    e7 -- psum-bank recycling is then paced by the 652ns relus (an
    8-wide wave stalls ~2.3us waiting for relu0 on the 9th alloc).
  - Tensor p-state ramps 1.54 -> 0.83 -> 0.42 ns/cycle over ~3us of
    continuous busy and RESETS on a >~1.1us idle gap (a ~0.6-0.9us
    gap is survivable).  18 warmup matmuls on a memset tile bridge
    preamble-end (~7.3us) to data-ready (~11.2-11.9us, run-to-run
    DMA variance) so MM1 opens at full clock.
  - Writeback: out DMAs drain at ~30-45ns per descriptor per queue
    (descriptor = one partition row), so every piece is split by
    partitions 63/62 across the two HWDGE rings, and the last s-block
    tapers its c-chunks (500, 250, 250) -- LDWEIGHTS pipelines >=2
    deep so 250-row matmuls still stream at full rate -- halving the
    bytes behind the final completion semaphore.  gpsimd (SWDGE) was
    measured ~3.3us issue-to-semaphore and stays off the tail.
"""

import os

import numpy as np
import ml_dtypes

BF16 = ml_dtypes.bfloat16
FP8 = ml_dtypes.float8_e4m3

MODE = os.environ.get("BASS_KERNEL_MODE", "fp8")    # "bf16" | "fp8"
W1_SCALE = 32.0                                     # fp8 mode: keep W1 out of subnormals
N_WARM = int(os.environ.get("BASS_KERNEL_WARMUP", "20"))
WARM_W = 256                                        # warmup matmul moving dim

N_CORES = 8
T, D, E, C, J = 60000, 1024, 2048, 1000, 30
T_LOC = T // N_CORES          # 7500 rows per core
S_LOC = T_LOC // J            # 250 segments per core
CHUNK = 510                   # t-chunk (17 segs; psum row = 2040B, fits a
                              # 2KB bank); 14 x 510 + 360 -> one fewer
                              # chunk's worth of matmul/DMA overhead vs 480
E_TILES = E // 128            # 16
S_BLK = 125                   # MM2 output rows per block (2 blocks)
C_BLK = 500                   # MM2 output cols per chunk (2 chunks)
HE = E // 2                   # 1024: W1 half-slab column width

_cache = {}


def _build(mode):
    from concourse import bacc, mybir
    from concourse.tile import TileContext

    f32 = mybir.dt.float32
    bf16 = mybir.dt.bfloat16
    fp8 = mybir.dt.float8e4
    in_dt = fp8 if mode == "fp8" else bf16
    KT = 4 if mode == "fp8" else 8          # accumulation steps per psum group
    perf = mybir.MatmulPerfMode.DoubleRow if mode == "fp8" else None

    nc = bacc.Bacc(trn_type="TRN2", target_bir_lowering=False, debug=False,
                   num_devices=N_CORES, num_swdge_queues=4)

    # x shard pre-chunked on the host to [chunk][p=128][d_tile][t] so each
    # chunk is one partition-contiguous DMA (3840B descriptors).  fp8 W1 is
    # prepacked to [kt][half][p][j][e'] so each (kt, half) slab is one DMA
    # with a single 2KB descriptor per partition (row kt*256+j*128+p,
    # col half*1024+e').
    xt_d = nc.declare_dram_parameter("xt", [D * T_LOC], in_dt, isOutput=False)
    if mode == "fp8":
        w1t_d = nc.declare_dram_parameter("w1t", [D * E], in_dt, isOutput=False)
    else:
        w1t_d = nc.declare_dram_parameter("w1t", [D, E], in_dt, isOutput=False)
    wct_d = nc.declare_dram_parameter("wct", [E, C], bf16, isOutput=False)
    out_d = nc.declare_dram_parameter("out", [S_LOC, C], bf16, isOutput=True)

    # t-chunks: 15 x 480 + 1 x 300
    chunks = []
    t0 = 0
    while t0 < T_LOC:
        w = min(CHUNK, T_LOC - t0)
        chunks.append((t0, w))
        t0 += w

    wct_v = wct_d[:, :].rearrange("(e p) c -> p e c", p=128)
    h = E // 2

    with TileContext(nc) as tc:
        with (
            tc.tile_pool(name="xin", bufs=3) as px,
            tc.tile_pool(name="wgt", bufs=1) as pw,
            tc.tile_pool(name="zrl", bufs=4) as pz,
        ):
            # PE warmup fodder: the memset is gpsimd's first instruction, so
            # the dummy matmuls can start right at preamble-end.
            if mode == "fp8":
                dmy = pw.tile([128, 2, WARM_W], fp8, tag="dmy", name="dmy")
            else:
                dmy = pw.tile([128, WARM_W], bf16, tag="dmy", name="dmy")
            nc.gpsimd.memset(dmy, 0)

            # --- W1 tiles ---
            if mode == "fp8":
                # per (kt, half): [128, 2 j, 1024] -- 2KB/partition on both
                # sides of the DMA, loaded in e-wave consumption order.
                # (Quarter-granularity was tried and measured slower: more
                # per-LDWEIGHTS semaphore waits without moving data-ready.)
                w1_sb = [[pw.tile([128, 2, HE], fp8, tag=f"w1_{k}_{hh}",
                                  name=f"w1_{k}_{hh}") for hh in range(2)]
                         for k in range(KT)]
            else:
                w1_sb = [pw.tile([128, E], bf16, tag=f"w1_{k}",
                                 name=f"w1_{k}") for k in range(KT)]

            def wpiece_bf16(eng, kt, piece):
                cs = slice(piece * h, (piece + 1) * h)
                eng.dma_start(out=w1_sb[kt][:, cs],
                              in_=w1t_d[kt * 128:(kt + 1) * 128, cs])

            def w1view(kt, hh):
                blk = 128 * 2 * HE
                base = (kt * 2 + hh) * blk
                return w1t_d[base:base + blk].rearrange(
                    "(p j e) -> p j e", p=128, j=2)

            wc_sb = pw.tile([128, E_TILES, C], bf16, tag="wc", name="wc")

            feats = [pw.tile([128, S_LOC], f32, tag=f"fs_{e}", name=f"fs_{e}")
                     for e in range(E_TILES)]

            def lhsT(kt, e):
                if mode == "fp8":
                    hh, e2 = divmod(e, 8)
                    return w1_sb[kt][hh][:, :, e2 * 128:(e2 + 1) * 128]
                return w1_sb[kt][:, e * 128:(e + 1) * 128]

            def rhs(xt, kt, w):
                if mode == "fp8":
                    return xt[:, 2 * kt:2 * kt + 2, :w]
                return xt[:, kt, :w]

            featsb = [pw.tile([128, S_LOC], bf16, tag=f"fb_{e}", name=f"fb_{e}")
                      for e in range(E_TILES)]

            def relu_pool(ps, w, e, s0, last=False):
                segs = w // J
                zr = pz.tile([128, CHUNK], f32, tag="zr", name="zr")
                nc.scalar.activation(zr[:, :w], ps[:, :w],
                                     mybir.ActivationFunctionType.Relu)
                nc.vector.tensor_reduce(
                    out=feats[e][:, s0:s0 + segs],
                    in_=zr[:, :w].rearrange("p (s j) -> p s j", j=J),
                    axis=mybir.AxisListType.X,
                    op=mybir.AluOpType.add,
                )
                if last:
                    # feats[e] is complete once the last chunk's pool ran;
                    # convert for MM2 right away so MM2 never waits.
                    nc.vector.tensor_copy(featsb[e], feats[e])

            n_dt = 2 * KT if mode == "fp8" else KT

            xt0 = px.tile([128, n_dt, CHUNK], in_dt, tag="xt", name="xt")
            cv0 = xt_d[0:D * CHUNK].rearrange("(p d t) -> p d t", p=128, d=n_dt)

            # ring bring-up is serialized and costs ~5ns per descriptor of
            # the ring's HEAD DMA, so a 1-descriptor micro-DMA brings each
            # HWDGE ring up cheaply before the real slabs queue.
            for nm, eng in (("sy", nc.sync), ("sc", nc.scalar)):
                scr = pw.tile([1, 128], in_dt, tag=f"scr_{nm}",
                              name=f"scr_{nm}")
                if mode == "fp8":
                    eng.dma_start(out=scr, in_=xt_d[0:128].rearrange(
                        "(p t) -> p t", p=1))
                else:
                    eng.dma_start(out=scr, in_=w1t_d[0:1, 0:128])

            if mode == "fp8":
                # sync: x chunk-0 split so the kt0 rhs lands first; the rest
                # of the chunk follows as one 2880B-descriptor DMA.  The h1
                # W1 slabs (e-wave 1, consumed from ~16us) follow on sync so
                # scalar only issues 4 DMAs and is free for relus by ~10.5us
                # (wave-1 PSUM-bank recycling is relu-paced).
                # Priming balance: the first matmul gates on kt0-h0 (both
                # j slabs) + x0's kt0 d-pair.  Those four pieces split 2+2
                # across the HWDGE queues (~190KB each) so the gate is
                # ~0.97us instead of kt0-h0's 1.3us on one queue.  Startup
                # DRAM->SBUF descriptors process at ~5-8ns, so the 510B/1KB
                # descriptors here are bandwidth- not descriptor-bound
                # (unlike the SBUF->DRAM tail).  h1 half-slabs (e8..15,
                # ~7us later) queue on sync BEHIND chunk-1's x.  Emission
                # order defines the dataflow graph, so everything chunk-0
                # reads is emitted here, before the chunk-0 matmuls.
                w00 = w1view(0, 0)
                nc.scalar.dma_start(out=w1_sb[0][0][:, 0, :], in_=w00[:, 0, :])
                nc.scalar.dma_start(out=xt0[:, 1:2, :], in_=cv0[:, 1:2, :])
                nc.scalar.dma_start(out=w1_sb[1][0], in_=w1view(1, 0))
                nc.scalar.dma_start(out=w1_sb[2][0], in_=w1view(2, 0))
                nc.sync.dma_start(out=xt0[:, 0:1, :], in_=cv0[:, 0:1, :])
                nc.sync.dma_start(out=w1_sb[0][0][:, 1, :], in_=w00[:, 1, :])
                nc.sync.dma_start(out=xt0[:, 2:4, :], in_=cv0[:, 2:4, :])
                nc.sync.dma_start(out=xt0[:, 4:n_dt, :], in_=cv0[:, 4:n_dt, :])
                nc.sync.dma_start(out=w1_sb[3][0], in_=w1view(3, 0))
                xt1 = px.tile([128, n_dt, CHUNK], in_dt, tag="xt", name="xt1")
                cv1 = xt_d[D * CHUNK:D * 2 * CHUNK].rearrange(
                    "(p d t) -> p d t", p=128, d=n_dt)
                nc.sync.dma_start(out=xt1, in_=cv1)
                for k in range(KT):
                    nc.sync.dma_start(out=w1_sb[k][1], in_=w1view(k, 1))
            else:
                nc.gpsimd.dma_start(out=xt0[:, 0:4, :], in_=cv0[:, 0:4, :])
                nc.gpsimd.dma_start(out=xt0[:, 4:8, :], in_=cv0[:, 4:8, :])
                for k in range(KT):
                    wpiece_bf16(nc.sync, k, 1)
                for k in range(KT):
                    wpiece_bf16(nc.scalar, k, 0)
                # bf16 fallback keeps the old single wct DMA on scalar.
                nc.scalar.dma_start(out=wc_sb, in_=wct_v)

            with tc.tile_pool(name="ps1", bufs=8, space="PSUM") as pp1:
                # warmup matmuls: ramp the PE p-state while the first real
                # weights/x stream in.  Results are never read.  Shares the
                # "ps" tag so the pool stays at 8 banks.
                wps = pp1.tile([128, CHUNK], f32, tag="ps", name="warm")
                for _ in range(N_WARM):
                    if mode == "fp8":
                        nc.tensor.matmul(wps[:, :WARM_W], dmy[:, :, :128], dmy,
                                         start=True, stop=True, perf_mode=perf)
                    else:
                        nc.tensor.matmul(wps[:, :WARM_W], dmy[:, :128], dmy,
                                         start=True, stop=True)

                for ci, (t0, w) in enumerate(chunks):
                    if ci == 0:
                        xt = xt0
                    elif ci == 1 and mode == "fp8":
                        xt = xt1
                    else:
                        xt = px.tile([128, n_dt, CHUNK], in_dt, tag="xt",
                                     name="xt")
                        cv = xt_d[D * t0:D * (t0 + w)].rearrange(
                            "(p d t) -> p d t", p=128, d=n_dt)
                        nc.sync.dma_start(out=xt[:, :, :w], in_=cv)
                        if mode == "fp8" and ci == 4:
                            # MM2 weights: one big DMA queued on sync BEHIND
                            # chunk-4's x -- the xt pool-slot gating delays
                            # its transfer past the startup-critical window
                            # without touching scalar (busy with relus).
                            nc.sync.dma_start(out=wc_sb, in_=wct_v)
                    s0 = t0 // J
                    if ci == 0 and mode == "fp8":
                        # wave 0 (e0..5): k-outer across 6 parallel psum
                        # groups -- the first MMs only need the kt0 h0
                        # half-slab + the first x k-slices, and each kt row
                        # (1.2us) covers the next half-slab's delivery.
                        # e7..15 then go k-inner (one psum at a time): bank 0
                        # (warmup tile) is free for e7, and from e8 on the
                        # recycled bank's relu (started back at wave-0 kt3)
                        # is already done -- an 8-wide wave instead stalls
                        # ~2.3us waiting for relu0 on the 9th psum alloc.
                        wave = 7
                        pss = [pp1.tile([128, CHUNK], f32, tag="ps",
                                        name=f"ps0_{i}") for i in range(wave)]
                        for kt in range(KT):
                            for i in range(wave):
                                nc.tensor.matmul(
                                    pss[i][:, :w],
                                    lhsT(kt, i),
                                    rhs(xt, kt, w),
                                    start=(kt == 0),
                                    stop=(kt == KT - 1),
                                    perf_mode=perf,
                                )
                        for i in range(wave):
                            relu_pool(pss[i], w, i, s0)
                        e_range = range(wave, E_TILES)
                    elif ci == 0:
                        # bf16 fallback: original two k-outer waves.
                        e0 = 0
                        for wave in (8, 8):
                            pss = [pp1.tile([128, CHUNK], f32, tag="ps",
                                            name=f"ps0_{e0}_{i}")
                                   for i in range(wave)]
                            for kt in range(KT):
                                for i in range(wave):
                                    nc.tensor.matmul(
                                        pss[i][:, :w],
                                        lhsT(kt, e0 + i),
                                        rhs(xt, kt, w),
                                        start=(kt == 0),
                                        stop=(kt == KT - 1),
                                        perf_mode=perf,
                                    )
                            for i in range(wave):
                                relu_pool(pss[i], w, e0 + i, s0)
                            e0 += wave
                        continue
                    else:
                        e_range = range(E_TILES)
                    for e in e_range:
                        ps = pp1.tile([128, CHUNK], f32, tag="ps", name="ps")
                        for kt in range(KT):
                            nc.tensor.matmul(
                                ps[:, :w],
                                lhsT(kt, e),
                                rhs(xt, kt, w),
                                start=(kt == 0),
                                stop=(kt == KT - 1),
                                perf_mode=perf,
                            )
                        relu_pool(ps, w, e, s0, last=(ci == len(chunks) - 1))

                # MM2 + writeback: all output DMAs ride the sync HWDGE ring
                # (idle by now).  The very last [125,500] copy+DMA is split
                # 375/125 so the final completion semaphore fires sooner.
                for sb in range(S_LOC // S_BLK):
                    ob = pw.tile([S_BLK, C], bf16, tag=f"ob_{sb}", name=f"ob_{sb}")
                    rows = slice(sb * S_BLK, (sb + 1) * S_BLK)
                    # last s-block tapers its c-chunks (LDWEIGHTS pipelines
                    # >=2 deep, so 250-row matmuls still stream at full
                    # rate): the final writeback piece is half the bytes,
                    # and ~45 B/ns per ring makes the tail byte-bound.
                    cchunks = ((0, C_BLK), (C_BLK, C_BLK)) \
                        if sb < S_LOC // S_BLK - 1 else \
                        ((0, C_BLK), (C_BLK, 250), (C_BLK + 250, 250))
                    for c0, cw in cchunks:
                        ps = pp1.tile([S_BLK, C_BLK], f32, tag="ps", name="ps2")
                        for e in range(E_TILES):
                            nc.tensor.matmul(
                                ps[:, :cw],
                                featsb[e][:, rows],
                                wc_sb[:, e, c0:c0 + cw],
                                start=(e == 0),
                                stop=(e == E_TILES - 1),
                            )
                        # writeback split by PARTITIONS across the two HWDGE
                        # rings: a ring moves ~45 B/ns here (2 DMA engines x
                        # 22.5 B/ns), so one 125KB piece holds its queue for
                        # ~2.8us and head-of-line-blocks the final piece.
                        # 63+62 rows per ring keeps both queues drained and
                        # the last completion semaphore fires ~1.5us sooner.
                        cs = slice(c0, c0 + cw)
                        r0 = sb * S_BLK
                        nc.scalar.copy(ob[:, cs], ps[:, :cw])
                        if (sb, c0) == (S_LOC // S_BLK - 1, C_BLK):
                            # second-to-last piece rides the otherwise-idle
                            # gpsimd SWDGE queue (its ~3-4us latency hides
                            # under the final block's matmuls) so both
                            # HWDGE rings are EMPTY when the final piece's
                            # descriptors arrive -- the drain is bound at
                            # ~45ns per descriptor per queue, so HoL
                            # blocking there directly delays the barrier.
                            nc.gpsimd.dma_start(out=out_d[rows, cs],
                                                in_=ob[:, cs])
                        else:
                            nc.sync.dma_start(
                                out=out_d[r0:r0 + 63, cs], in_=ob[0:63, cs])
                            nc.scalar.dma_start(
                                out=out_d[r0 + 63:r0 + S_BLK, cs],
                                in_=ob[63:S_BLK, cs])

    nc.compile()
    return nc


def _prep_inputs(x, W1, Wlin, mode=MODE):
    wc = (Wlin[:, :E] + Wlin[:, E:]) / np.float32(J)     # [C, E] f32
    if mode == "fp8":
        in_np = FP8
        W1 = W1 * np.float32(W1_SCALE)
        wc = wc / np.float32(W1_SCALE)
    else:
        in_np = BF16
    wct = np.ascontiguousarray(wc.T).astype(BF16)        # [E, C] bf16
    if mode == "fp8":
        # prepack to [kt][half][p][j][e']: row kt*256 + j*128 + p,
        # col half*1024 + e'  (one 2KB descriptor per partition per slab)
        KT = 4
        a = W1.T.reshape(KT, 2, 128, 2, HE)              # [kt][j][p][h][e']
        w1t = np.ascontiguousarray(
            a.transpose(0, 3, 2, 1, 4)).astype(in_np).ravel()
    else:
        w1t = np.ascontiguousarray(W1.T).astype(in_np)   # [D, E]
    in_maps = []
    for c in range(N_CORES):
        xs = x[c * T_LOC:(c + 1) * T_LOC]                # [7500, 1024]
        pieces = []
        t0 = 0
        while t0 < T_LOC:                                # [p][d_tile][t] chunks
            w = min(CHUNK, T_LOC - t0)
            blk = xs[t0:t0 + w].T.reshape(8, 128, w).transpose(1, 0, 2)
            pieces.append(np.ascontiguousarray(blk).astype(in_np).ravel())
            t0 += w
        xt = np.concatenate(pieces)                      # [D*T_LOC] flat
        in_maps.append({"xt": xt, "w1t": w1t, "wct": wct})
    return in_maps


def _run(in_maps, mode=MODE, trace=False, **kw):
    from concourse.bass_utils import run_bass_kernel_spmd

    if mode not in _cache:
        _cache[mode] = _build(mode)
    res = run_bass_kernel_spmd(_cache[mode], in_maps,
                               core_ids=list(range(N_CORES)), trace=trace, **kw)
    logits = np.concatenate(
        [np.asarray(r["out"]) for r in res.results], axis=0).astype(np.float32)
    return logits, res


def kernel(x, y, W1, W2, Wlin):
    x = np.asarray(x, dtype=np.float32)
    W1 = np.asarray(W1, dtype=np.float32)
    Wlin = np.asarray(Wlin, dtype=np.float32)
    modes = (MODE, "bf16") if MODE != "bf16" else ("bf16",)
    for i, mode in enumerate(modes):
        try:
            logits, _ = _run(_prep_inputs(x, W1, Wlin, mode=mode), mode=mode)
            return logits
        except Exception:
            if i == len(modes) - 1:
                raise
    raise RuntimeError("unreachable")


# revision 40
# speedup vs baseline: 1.1873x; 1.1873x over previous
"""Trainium2 Bass kernel for nn_Classifier_69818988363910 (segment_reduce).

Reference computation (after dead-code elimination):
    local = relu(x @ W1^T)                        # [60000, 2048]
    feats = local.reshape(2000, 30, 2048).mean(1) # [2000, 2048]
    logits = concat(feats, feats) @ Wlin^T        # [2000, 1000]
           = feats @ (Wlin[:, :2048] + Wlin[:, 2048:])^T
y / W2 are computed but unused in the reference (original-code bug), so the
output depends only on x, W1, Wlin.

Sharding: data-parallel over the 8 NeuronCores along T (7500 rows = 250
segments per core); W1 / Wc replicated. No collectives; host gathers.

Device kernel per core (fp32 accumulation in PSUM throughout):
    MM1 on PE:   z[e, t] = sum_d W1T[d, e] * xT[d, t]
                 bf16 mode: 8 k-tiles of 128;  fp8 mode: 4 DoubleRow
                 super-k-tiles of 256 (2x PE rate)
    relu on ACT: psum -> sbuf
    pool on DVE: tensor_reduce over [128, segs, 30] view (sum; the 1/30
                 mean scale and the fp8 W1 pre-scale are folded into Wc
                 on the host)
    MM2 on PE:   logits[s, c] = sum_e featsT[e, s] * WcT[e, c]  (bf16)

Perf notes (trace-driven; staged baseline 237.2us -> ~235.1-236.0us):
  - The PE runs MM1 at the fp8-DoubleRow peak (202ns per 256x128x480
    matmul, 0.42ns per moving row); MM1 202.7us + MM2 13.4us of PE
    stream is the hard floor.  This version runs the stream with
    <0.7us of total stalls; the rest is startup + writeback tail.
  - Framework preamble ends ~6.9us; DMA transfers can only start
    ~8.5us (issue + ring bring-up), and each DMA's completion
    semaphore costs ~900ns after the data lands.  Only sync (SP) and
    scalar (ACT) have HWDGE rings; total fabric ~390 B/ns, one ring
    can pull ~195 B/ns while the other is active.
  - Startup schedule (first real matmul gates on kt0-h0 + x0-kt0pair,
    one ~125-256KB piece per HWDGE queue):
      scalar: micro, W1 h0 half-slabs kt0, kt1 (prepacked
              [kt][half][p][j][e'] -> one 2KB descriptor/partition).
      sync:   micro, x0(d0:2), x0(d2:4), kt2-h0, x0(d4:8), kt3-h0,
              x1, then the h1 half-slabs (needed only from e8,
              ~7us after open), then x chunks 2..15 with the 4MB wct
              DMA behind chunk-4's x (pool-slot gating pushes its
              transfer past the startup window; needed ~215us in).
    Tile builds the dependency graph from EMISSION order, so chunk-1's
    x and the h1 slabs are emitted before the chunk-0 matmul bodies
    that must wait on them.
  - Chunk 0 compute: kt-outer wave over e0..6 (each 1.4us kt row
    covers the next half-slab's delivery), then kt-inner per e from
    e7 -- psum-bank recycling is then paced by the 652ns relus (an
    8-wide wave stalls ~2.3us waiting for relu0 on the 9th alloc).
  - Tensor p-state ramps 1.54 -> 0.83 -> 0.42 ns/cycle over ~3us of
    continuous busy and RESETS on a >~1.1us idle gap (a ~0.6-0.9us
    gap is survivable).  18 warmup matmuls on a memset tile bridge
    preamble-end (~7.3us) to data-ready (~11.2-11.9us, run-to-run
    DMA variance) so MM1 opens at full clock.
  - Writeback: out DMAs drain at ~30-45ns per descriptor per queue
    (descriptor = one partition row), so every piece is split by
    partitions 63/62 across the two HWDGE rings, and the last s-block
    tapers its c-chunks (500, 250, 250) -- LDWEIGHTS pipelines >=2
    deep so 250-row matmuls still stream at full rate -- halving the
    bytes behind the final completion semaphore.  gpsimd (SWDGE) was
    measured ~3.3us issue-to-semaphore and stays off the tail.
"""

import os

import numpy as np
import ml_dtypes

BF16 = ml_dtypes.bfloat16
FP8 = ml_dtypes.float8_e4m3

MODE = os.environ.get("BASS_KERNEL_MODE", "fp8")    # "bf16" | "fp8"
W1_SCALE = 32.0                                     # fp8 mode: keep W1 out of subnormals
N_WARM = int(os.environ.get("BASS_KERNEL_WARMUP", "20"))
WARM_W = 256                                        # warmup matmul moving dim

N_CORES = 8
T, D, E, C, J = 60000, 1024, 2048, 1000, 30
T_LOC = T // N_CORES          # 7500 rows per core
S_LOC = T_LOC // J            # 250 segments per core
CHUNK = 510                   # t-chunk (17 segs; psum row = 2040B, fits a
                              # 2KB bank); 14 x 510 + 360 -> one fewer
                              # chunk's worth of matmul/DMA overhead vs 480
E_TILES = E // 128            # 16
S_BLK = 125                   # MM2 output rows per block (2 blocks)
C_BLK = 500                   # MM2 output cols per chunk (2 chunks)
HE = E // 2                   # 1024: W1 half-slab column width

_cache = {}


def _build(mode):
    from concourse import bacc, mybir
    from concourse.tile import TileContext

    f32 = mybir.dt.float32
    bf16 = mybir.dt.bfloat16
    fp8 = mybir.dt.float8e4
    in_dt = fp8 if mode == "fp8" else bf16
    KT = 4 if mode == "fp8" else 8          # accumulation steps per psum group
    perf = mybir.MatmulPerfMode.DoubleRow if mode == "fp8" else None

    nc = bacc.Bacc(trn_type="TRN2", target_bir_lowering=False, debug=False,
                   num_devices=N_CORES, num_swdge_queues=4)

    # x shard pre-chunked on the host to [chunk][p=128][d_tile][t] so each
    # chunk is one partition-contiguous DMA (3840B descriptors).  fp8 W1 is
    # prepacked to [kt][half][p][j][e'] so each (kt, half) slab is one DMA
    # with a single 2KB descriptor per partition (row kt*256+j*128+p,
    # col half*1024+e').
    xt_d = nc.declare_dram_parameter("xt", [D * T_LOC], in_dt, isOutput=False)
    if mode == "fp8":
        w1t_d = nc.declare_dram_parameter("w1t", [D * E], in_dt, isOutput=False)
    else:
        w1t_d = nc.declare_dram_parameter("w1t", [D, E], in_dt, isOutput=False)
    wct_d = nc.declare_dram_parameter("wct", [E, C], bf16, isOutput=False)
    out_d = nc.declare_dram_parameter("out", [S_LOC, C], bf16, isOutput=True)

    # t-chunks: 15 x 480 + 1 x 300
    chunks = []
    t0 = 0
    while t0 < T_LOC:
        w = min(CHUNK, T_LOC - t0)
        chunks.append((t0, w))
        t0 += w

    wct_v = wct_d[:, :].rearrange("(e p) c -> p e c", p=128)
    h = E // 2

    with TileContext(nc) as tc:
        with (
            tc.tile_pool(name="xin", bufs=3) as px,
            tc.tile_pool(name="wgt", bufs=1) as pw,
            tc.tile_pool(name="zrl", bufs=4) as pz,
        ):
            # PE warmup fodder: the memset is gpsimd's first instruction, so
            # the dummy matmuls can start right at preamble-end.
            if mode == "fp8":
                dmy = pw.tile([128, 2, WARM_W], fp8, tag="dmy", name="dmy")
            else:
                dmy = pw.tile([128, WARM_W], bf16, tag="dmy", name="dmy")
            nc.gpsimd.memset(dmy, 0)

            # --- W1 tiles ---
            if mode == "fp8":
                # per (kt, half): [128, 2 j, 1024] -- 2KB/partition on both
                # sides of the DMA, loaded in e-wave consumption order.
                # (Quarter-granularity was tried and measured slower: more
                # per-LDWEIGHTS semaphore waits without moving data-ready.)
                w1_sb = [[pw.tile([128, 2, HE], fp8, tag=f"w1_{k}_{hh}",
                                  name=f"w1_{k}_{hh}") for hh in range(2)]
                         for k in range(KT)]
            else:
                w1_sb = [pw.tile([128, E], bf16, tag=f"w1_{k}",
                                 name=f"w1_{k}") for k in range(KT)]

            def wpiece_bf16(eng, kt, piece):
                cs = slice(piece * h, (piece + 1) * h)
                eng.dma_start(out=w1_sb[kt][:, cs],
                              in_=w1t_d[kt * 128:(kt + 1) * 128, cs])

            def w1view(kt, hh):
                blk = 128 * 2 * HE
                base = (kt * 2 + hh) * blk
                return w1t_d[base:base + blk].rearrange(
                    "(p j e) -> p j e", p=128, j=2)

            wc_sb = pw.tile([128, E_TILES, C], bf16, tag="wc", name="wc")

            feats = [pw.tile([128, S_LOC], f32, tag=f"fs_{e}", name=f"fs_{e}")
                     for e in range(E_TILES)]

            def lhsT(kt, e):
                if mode == "fp8":
                    hh, e2 = divmod(e, 8)
                    return w1_sb[kt][hh][:, :, e2 * 128:(e2 + 1) * 128]
                return w1_sb[kt][:, e * 128:(e + 1) * 128]

            def rhs(xt, kt, w):
                if mode == "fp8":
                    return xt[:, 2 * kt:2 * kt + 2, :w]
                return xt[:, kt, :w]

            featsb = [pw.tile([128, S_LOC], bf16, tag=f"fb_{e}", name=f"fb_{e}")
                      for e in range(E_TILES)]

            def relu_pool(ps, w, e, s0, last=False):
                segs = w // J
                zr = pz.tile([128, CHUNK], f32, tag="zr", name="zr")
                nc.scalar.activation(zr[:, :w], ps[:, :w],
                                     mybir.ActivationFunctionType.Relu)
                nc.vector.tensor_reduce(
                    out=feats[e][:, s0:s0 + segs],
                    in_=zr[:, :w].rearrange("p (s j) -> p s j", j=J),
                    axis=mybir.AxisListType.X,
                    op=mybir.AluOpType.add,
                )
                if last:
                    # feats[e] is complete once the last chunk's pool ran;
                    # convert for MM2 right away so MM2 never waits.
                    nc.vector.tensor_copy(featsb[e], feats[e])

            n_dt = 2 * KT if mode == "fp8" else KT

            xt0 = px.tile([128, n_dt, CHUNK], in_dt, tag="xt", name="xt")
            cv0 = xt_d[0:D * CHUNK].rearrange("(p d t) -> p d t", p=128, d=n_dt)

            # ring bring-up is serialized and costs ~5ns per descriptor of
            # the ring's HEAD DMA, so a 1-descriptor micro-DMA brings each
            # HWDGE ring up cheaply before the real slabs queue.
            for nm, eng in (("sy", nc.sync), ("sc", nc.scalar)):
                scr = pw.tile([1, 128], in_dt, tag=f"scr_{nm}",
                              name=f"scr_{nm}")
                if mode == "fp8":
                    eng.dma_start(out=scr, in_=xt_d[0:128].rearrange(
                        "(p t) -> p t", p=1))
                else:
                    eng.dma_start(out=scr, in_=w1t_d[0:1, 0:128])

            if mode == "fp8":
                # sync: x chunk-0 split so the kt0 rhs lands first; the rest
                # of the chunk follows as one 2880B-descriptor DMA.  The h1
                # W1 slabs (e-wave 1, consumed from ~16us) follow on sync so
                # scalar only issues 4 DMAs and is free for relus by ~10.5us
                # (wave-1 PSUM-bank recycling is relu-paced).
                # Priming balance: the first matmul gates on (x0a | kt0-h0),
                # one ~125KB piece per HWDGE queue, so scalar carries only
                # kt0/kt1 h0 and sync interleaves kt2/kt3 h0 into its x0
                # stream in consumption order.  h1 half-slabs (e8..15, ~7us
                # later) queue on sync BEHIND chunk-1's x.  Emission order
                # defines the dataflow graph, so everything chunk-0 reads is
                # emitted here, before the chunk-0 matmuls.
                # kt0-h0's two j slabs and x0's kt0 d-pair split 2+2 across
                # the queues (~190KB each) so the priming gate is ~0.97us
                # instead of kt0-h0's 1.3us on one queue; startup DRAM->SBUF
                # descriptors process at ~5-8ns so the 510B/1KB descriptors
                # stay bandwidth-bound.
                w00 = w1view(0, 0)
                nc.scalar.dma_start(out=w1_sb[0][0][:, 0, :], in_=w00[:, 0, :])
                nc.scalar.dma_start(out=xt0[:, 1:2, :], in_=cv0[:, 1:2, :])
                nc.scalar.dma_start(out=w1_sb[1][0], in_=w1view(1, 0))
                nc.scalar.dma_start(out=w1_sb[2][0], in_=w1view(2, 0))
                nc.sync.dma_start(out=xt0[:, 0:1, :], in_=cv0[:, 0:1, :])
                nc.sync.dma_start(out=w1_sb[0][0][:, 1, :], in_=w00[:, 1, :])
                nc.sync.dma_start(out=xt0[:, 2:4, :], in_=cv0[:, 2:4, :])
                nc.sync.dma_start(out=xt0[:, 4:n_dt, :], in_=cv0[:, 4:n_dt, :])
                nc.sync.dma_start(out=w1_sb[3][0], in_=w1view(3, 0))
                xt1 = px.tile([128, n_dt, CHUNK], in_dt, tag="xt", name="xt1")
                cv1 = xt_d[D * CHUNK:D * 2 * CHUNK].rearrange(
                    "(p d t) -> p d t", p=128, d=n_dt)
                nc.sync.dma_start(out=xt1, in_=cv1)
                for k in range(KT):
                    nc.sync.dma_start(out=w1_sb[k][1], in_=w1view(k, 1))
            else:
                nc.gpsimd.dma_start(out=xt0[:, 0:4, :], in_=cv0[:, 0:4, :])
                nc.gpsimd.dma_start(out=xt0[:, 4:8, :], in_=cv0[:, 4:8, :])
                for k in range(KT):
                    wpiece_bf16(nc.sync, k, 1)
                for k in range(KT):
                    wpiece_bf16(nc.scalar, k, 0)
                # bf16 fallback keeps the old single wct DMA on scalar.
                nc.scalar.dma_start(out=wc_sb, in_=wct_v)

            with tc.tile_pool(name="ps1", bufs=8, space="PSUM") as pp1:
                # warmup matmuls: ramp the PE p-state while the first real
                # weights/x stream in.  Results are never read.  Shares the
                # "ps" tag so the pool stays at 8 banks.
                wps = pp1.tile([128, CHUNK], f32, tag="ps", name="warm")
                for _ in range(N_WARM):
                    if mode == "fp8":
                        nc.tensor.matmul(wps[:, :WARM_W], dmy[:, :, :128], dmy,
                                         start=True, stop=True, perf_mode=perf)
                    else:
                        nc.tensor.matmul(wps[:, :WARM_W], dmy[:, :128], dmy,
                                         start=True, stop=True)

                for ci, (t0, w) in enumerate(chunks):
                    if ci == 0:
                        xt = xt0
                    elif ci == 1 and mode == "fp8":
                        xt = xt1
                    else:
                        xt = px.tile([128, n_dt, CHUNK], in_dt, tag="xt",
                                     name="xt")
                        cv = xt_d[D * t0:D * (t0 + w)].rearrange(
                            "(p d t) -> p d t", p=128, d=n_dt)
                        nc.sync.dma_start(out=xt[:, :, :w], in_=cv)
                        if mode == "fp8" and ci == 4:
                            # MM2 weights: one big DMA queued on sync BEHIND
                            # chunk-4's x -- the xt pool-slot gating delays
                            # its transfer past the startup-critical window
                            # without touching scalar (busy with relus).
                            nc.sync.dma_start(out=wc_sb, in_=wct_v)
                    s0 = t0 // J
                    if ci == 0 and mode == "fp8":
                        # wave 0 (e0..5): k-outer across 6 parallel psum
                        # groups -- the first MMs only need the kt0 h0
                        # half-slab + the first x k-slices, and each kt row
                        # (1.2us) covers the next half-slab's delivery.
                        # e7..15 then go k-inner (one psum at a time): bank 0
                        # (warmup tile) is free for e7, and from e8 on the
                        # recycled bank's relu (started back at wave-0 kt3)
                        # is already done -- an 8-wide wave instead stalls
                        # ~2.3us waiting for relu0 on the 9th psum alloc.
                        wave = 7
                        pss = [pp1.tile([128, CHUNK], f32, tag="ps",
                                        name=f"ps0_{i}") for i in range(wave)]
                        for kt in range(KT):
                            for i in range(wave):
                                nc.tensor.matmul(
                                    pss[i][:, :w],
                                    lhsT(kt, i),
                                    rhs(xt, kt, w),
                                    start=(kt == 0),
                                    stop=(kt == KT - 1),
                                    perf_mode=perf,
                                )
                        for i in range(wave):
                            relu_pool(pss[i], w, i, s0)
                        e_range = range(wave, E_TILES)
                    elif ci == 0:
                        # bf16 fallback: original two k-outer waves.
                        e0 = 0
                        for wave in (8, 8):
                            pss = [pp1.tile([128, CHUNK], f32, tag="ps",
                                            name=f"ps0_{e0}_{i}")
                                   for i in range(wave)]
                            for kt in range(KT):
                                for i in range(wave):
                                    nc.tensor.matmul(
                                        pss[i][:, :w],
                                        lhsT(kt, e0 + i),
                                        rhs(xt, kt, w),
                                        start=(kt == 0),
                                        stop=(kt == KT - 1),
                                        perf_mode=perf,
                                    )
                            for i in range(wave):
                                relu_pool(pss[i], w, e0 + i, s0)
                            e0 += wave
                        continue
                    else:
                        e_range = range(E_TILES)
                    for e in e_range:
                        ps = pp1.tile([128, CHUNK], f32, tag="ps", name="ps")
                        for kt in range(KT):
                            nc.tensor.matmul(
                                ps[:, :w],
                                lhsT(kt, e),
                                rhs(xt, kt, w),
                                start=(kt == 0),
                                stop=(kt == KT - 1),
                                perf_mode=perf,
                            )
                        relu_pool(ps, w, e, s0, last=(ci == len(chunks) - 1))

                # MM2 + writeback: all output DMAs ride the sync HWDGE ring
                # (idle by now).  The very last [125,500] copy+DMA is split
                # 375/125 so the final completion semaphore fires sooner.
                for sb in range(S_LOC // S_BLK):
                    ob = pw.tile([S_BLK, C], bf16, tag=f"ob_{sb}", name=f"ob_{sb}")
                    rows = slice(sb * S_BLK, (sb + 1) * S_BLK)
                    # last s-block tapers its c-chunks (LDWEIGHTS pipelines
                    # >=2 deep, so 250-row matmuls still stream at full
                    # rate): the final writeback piece is half the bytes,
                    # and ~45 B/ns per ring makes the tail byte-bound.
                    cchunks = ((0, C_BLK), (C_BLK, C_BLK)) \
                        if sb < S_LOC // S_BLK - 1 else \
                        ((0, C_BLK), (C_BLK, 250), (C_BLK + 250, 250))
                    for c0, cw in cchunks:
                        ps = pp1.tile([S_BLK, C_BLK], f32, tag="ps", name="ps2")
                        for e in range(E_TILES):
                            nc.tensor.matmul(
                                ps[:, :cw],
                                featsb[e][:, rows],
                                wc_sb[:, e, c0:c0 + cw],
                                start=(e == 0),
                                stop=(e == E_TILES - 1),
                            )
                        # writeback split by PARTITIONS across the two HWDGE
                        # rings: a ring moves ~45 B/ns here (2 DMA engines x
                        # 22.5 B/ns), so one 125KB piece holds its queue for
                        # ~2.8us and head-of-line-blocks the final piece.
                        # 63+62 rows per ring keeps both queues drained and
                        # the last completion semaphore fires ~1.5us sooner.
                        cs = slice(c0, c0 + cw)
                        r0 = sb * S_BLK
                        nc.scalar.copy(ob[:, cs], ps[:, :cw])
                        if (sb, c0) == (S_LOC // S_BLK - 1, C_BLK):
                            # second-to-last piece rides the otherwise-idle
                            # gpsimd SWDGE queue (its ~3-4us latency hides
                            # under the final block's matmuls) so both
                            # HWDGE rings are EMPTY when the final piece's
                            # descriptors arrive -- the drain is bound at
                            # ~45ns per descriptor per queue, so HoL
                            # blocking there directly delays the barrier.
                            nc.gpsimd.dma_start(out=out_d[rows, cs],
                                                in_=ob[:, cs])
                        else:
                            nc.sync.dma_start(
                                out=out_d[r0:r0 + 63, cs], in_=ob[0:63, cs])
                            nc.scalar.dma_start(
                                out=out_d[r0 + 63:r0 + S_BLK, cs],
                                in_=ob[63:S_BLK, cs])

    nc.compile()
    return nc


def _prep_inputs(x, W1, Wlin, mode=MODE):
    wc = (Wlin[:, :E] + Wlin[:, E:]) / np.float32(J)     # [C, E] f32
    if mode == "fp8":
        in_np = FP8
        W1 = W1 * np.float32(W1_SCALE)
        wc = wc / np.float32(W1_SCALE)
    else:
        in_np = BF16
    wct = np.ascontiguousarray(wc.T).astype(BF16)        # [E, C] bf16
    if mode == "fp8":
        # prepack to [kt][half][p][j][e']: row kt*256 + j*128 + p,
        # col half*1024 + e'  (one 2KB descriptor per partition per slab)
        KT = 4
        a = W1.T.reshape(KT, 2, 128, 2, HE)              # [kt][j][p][h][e']
        w1t = np.ascontiguousarray(
            a.transpose(0, 3, 2, 1, 4)).astype(in_np).ravel()
    else:
        w1t = np.ascontiguousarray(W1.T).astype(in_np)   # [D, E]
    in_maps = []
    for c in range(N_CORES):
        xs = x[c * T_LOC:(c + 1) * T_LOC]                # [7500, 1024]
        pieces = []
        t0 = 0
        while t0 < T_LOC:                                # [p][d_tile][t] chunks
            w = min(CHUNK, T_LOC - t0)
            blk = xs[t0:t0 + w].T.reshape(8, 128, w).transpose(1, 0, 2)
            pieces.append(np.ascontiguousarray(blk).astype(in_np).ravel())
            t0 += w
        xt = np.concatenate(pieces)                      # [D*T_LOC] flat
        in_maps.append({"xt": xt, "w1t": w1t, "wct": wct})
    return in_maps


def _run(in_maps, mode=MODE, trace=False, **kw):
    from concourse.bass_utils import run_bass_kernel_spmd

    if mode not in _cache:
        _cache[mode] = _build(mode)
    res = run_bass_kernel_spmd(_cache[mode], in_maps,
                               core_ids=list(range(N_CORES)), trace=trace, **kw)
    logits = np.concatenate(
        [np.asarray(r["out"]) for r in res.results], axis=0).astype(np.float32)
    return logits, res


def kernel(x, y, W1, W2, Wlin):
    x = np.asarray(x, dtype=np.float32)
    W1 = np.asarray(W1, dtype=np.float32)
    Wlin = np.asarray(Wlin, dtype=np.float32)
    modes = (MODE, "bf16") if MODE != "bf16" else ("bf16",)
    for i, mode in enumerate(modes):
        try:
            logits, _ = _run(_prep_inputs(x, W1, Wlin, mode=mode), mode=mode)
            return logits
        except Exception:
            if i == len(modes) - 1:
                raise
    raise RuntimeError("unreachable")


# revision 41
# speedup vs baseline: 1.1947x; 1.0063x over previous
"""Trainium2 Bass kernel for nn_Classifier_69818988363910 (segment_reduce).

Reference computation (after dead-code elimination):
    local = relu(x @ W1^T)                        # [60000, 2048]
    feats = local.reshape(2000, 30, 2048).mean(1) # [2000, 2048]
    logits = concat(feats, feats) @ Wlin^T        # [2000, 1000]
           = feats @ (Wlin[:, :2048] + Wlin[:, 2048:])^T
y / W2 are computed but unused in the reference (original-code bug), so the
output depends only on x, W1, Wlin.

Sharding: data-parallel over the 8 NeuronCores along T (7500 rows = 250
segments per core); W1 / Wc replicated. No collectives; host gathers.

Device kernel per core (fp32 accumulation in PSUM throughout):
    MM1 on PE:   z[e, t] = sum_d W1T[d, e] * xT[d, t]
                 bf16 mode: 8 k-tiles of 128;  fp8 mode: 4 DoubleRow
                 super-k-tiles of 256 (2x PE rate)
    relu on ACT: psum -> sbuf
    pool on DVE: tensor_reduce over [128, segs, 30] view (sum; the 1/30
                 mean scale and the fp8 W1 pre-scale are folded into Wc
                 on the host)
    MM2 on PE:   logits[s, c] = sum_e featsT[e, s] * WcT[e, c]  (bf16)

Perf notes (trace-driven; staged baseline 237.2us -> ~235.1-236.0us):
  - The PE runs MM1 at the fp8-DoubleRow peak (202ns per 256x128x480
    matmul, 0.42ns per moving row); MM1 202.7us + MM2 13.4us of PE
    stream is the hard floor.  This version runs the stream with
    <0.7us of total stalls; the rest is startup + writeback tail.
  - Framework preamble ends ~6.9us; DMA transfers can only start
    ~8.5us (issue + ring bring-up), and each DMA's completion
    semaphore costs ~900ns after the data lands.  Only sync (SP) and
    scalar (ACT) have HWDGE rings; total fabric ~390 B/ns, one ring
    can pull ~195 B/ns while the other is active.
  - Startup schedule (first real matmul gates on kt0-h0 + x0-kt0pair,
    one ~125-256KB piece per HWDGE queue):
      scalar: micro, W1 h0 half-slabs kt0, kt1 (prepacked
              [kt][half][p][j][e'] -> one 2KB descriptor/partition).
      sync:   micro, x0(d0:2), x0(d2:4), kt2-h0, x0(d4:8), kt3-h0,
              x1, then the h1 half-slabs (needed only from e8,
              ~7us after open), then x chunks 2..15 with the 4MB wct
              DMA behind chunk-4's x (pool-slot gating pushes its
              transfer past the startup window; needed ~215us in).
    Tile builds the dependency graph from EMISSION order, so chunk-1's
    x and the h1 slabs are emitted before the chunk-0 matmul bodies
    that must wait on them.
  - Chunk 0 compute: kt-outer wave over e0..6 (each 1.4us kt row
    covers the next half-slab's delivery), then kt-inner per e from
    e7 -- psum-bank recycling is then paced by the 652ns relus (an
    8-wide wave stalls ~2.3us waiting for relu0 on the 9th alloc).
  - Tensor p-state ramps 1.54 -> 0.83 -> 0.42 ns/cycle over ~3us of
    continuous busy and RESETS on a >~1.1us idle gap (a ~0.6-0.9us
    gap is survivable).  18 warmup matmuls on a memset tile bridge
    preamble-end (~7.3us) to data-ready (~11.2-11.9us, run-to-run
    DMA variance) so MM1 opens at full clock.
  - Writeback: out DMAs drain at ~30-45ns per descriptor per queue
    (descriptor = one partition row), so every piece is split by
    partitions 63/62 across the two HWDGE rings, and the last s-block
    tapers its c-chunks (500, 250, 250) -- LDWEIGHTS pipelines >=2
    deep so 250-row matmuls still stream at full rate -- halving the
    bytes behind the final completion semaphore.  gpsimd (SWDGE) was
    measured ~3.3us issue-to-semaphore and stays off the tail.
"""

import os

import numpy as np
import ml_dtypes

BF16 = ml_dtypes.bfloat16
FP8 = ml_dtypes.float8_e4m3

MODE = os.environ.get("BASS_KERNEL_MODE", "fp8")    # "bf16" | "fp8"
W1_SCALE = 32.0                                     # fp8 mode: keep W1 out of subnormals
N_WARM = int(os.environ.get("BASS_KERNEL_WARMUP", "20"))
WARM_W = 256                                        # warmup matmul moving dim

N_CORES = 8
T, D, E, C, J = 60000, 1024, 2048, 1000, 30
T_LOC = T // N_CORES          # 7500 rows per core
S_LOC = T_LOC // J            # 250 segments per core
CHUNK = 510                   # t-chunk (17 segs; psum row = 2040B, fits a
                              # 2KB bank); 14 x 510 + 360 -> one fewer
                              # chunk's worth of matmul/DMA overhead vs 480
E_TILES = E // 128            # 16
S_BLK = 125                   # MM2 output rows per block (2 blocks)
C_BLK = 500                   # MM2 output cols per chunk (2 chunks)
HE = E // 2                   # 1024: W1 half-slab column width

_cache = {}


def _build(mode):
    from concourse import bacc, mybir
    from concourse.tile import TileContext

    f32 = mybir.dt.float32
    bf16 = mybir.dt.bfloat16
    fp8 = mybir.dt.float8e4
    in_dt = fp8 if mode == "fp8" else bf16
    KT = 4 if mode == "fp8" else 8          # accumulation steps per psum group
    perf = mybir.MatmulPerfMode.DoubleRow if mode == "fp8" else None

    nc = bacc.Bacc(trn_type="TRN2", target_bir_lowering=False, debug=False,
                   num_devices=N_CORES, num_swdge_queues=4)

    # x shard pre-chunked on the host to [chunk][p=128][d_tile][t] so each
    # chunk is one partition-contiguous DMA (3840B descriptors).  fp8 W1 is
    # prepacked to [kt][half][p][j][e'] so each (kt, half) slab is one DMA
    # with a single 2KB descriptor per partition (row kt*256+j*128+p,
    # col half*1024+e').
    xt_d = nc.declare_dram_parameter("xt", [D * T_LOC], in_dt, isOutput=False)
    if mode == "fp8":
        w1t_d = nc.declare_dram_parameter("w1t", [D * E], in_dt, isOutput=False)
    else:
        w1t_d = nc.declare_dram_parameter("w1t", [D, E], in_dt, isOutput=False)
    wct_d = nc.declare_dram_parameter("wct", [E, C], bf16, isOutput=False)
    out_d = nc.declare_dram_parameter("out", [S_LOC, C], bf16, isOutput=True)

    # t-chunks: 15 x 480 + 1 x 300
    chunks = []
    t0 = 0
    while t0 < T_LOC:
        w = min(CHUNK, T_LOC - t0)
        chunks.append((t0, w))
        t0 += w

    wct_v = wct_d[:, :].rearrange("(e p) c -> p e c", p=128)
    h = E // 2

    with TileContext(nc) as tc:
        with (
            tc.tile_pool(name="xin", bufs=3) as px,
            tc.tile_pool(name="wgt", bufs=1) as pw,
            tc.tile_pool(name="zrl", bufs=4) as pz,
        ):
            # PE warmup fodder: the memset is gpsimd's first instruction, so
            # the dummy matmuls can start right at preamble-end.
            if mode == "fp8":
                dmy = pw.tile([128, 2, WARM_W], fp8, tag="dmy", name="dmy")
            else:
                dmy = pw.tile([128, WARM_W], bf16, tag="dmy", name="dmy")
            nc.gpsimd.memset(dmy, 0)

            # --- W1 tiles ---
            if mode == "fp8":
                # per (kt, half): [128, 2 j, 1024] -- 2KB/partition on both
                # sides of the DMA, loaded in e-wave consumption order.
                # (Quarter-granularity was tried and measured slower: more
                # per-LDWEIGHTS semaphore waits without moving data-ready.)
                w1_sb = [[pw.tile([128, 2, HE], fp8, tag=f"w1_{k}_{hh}",
                                  name=f"w1_{k}_{hh}") for hh in range(2)]
                         for k in range(KT)]
            else:
                w1_sb = [pw.tile([128, E], bf16, tag=f"w1_{k}",
                                 name=f"w1_{k}") for k in range(KT)]

            def wpiece_bf16(eng, kt, piece):
                cs = slice(piece * h, (piece + 1) * h)
                eng.dma_start(out=w1_sb[kt][:, cs],
                              in_=w1t_d[kt * 128:(kt + 1) * 128, cs])

            def w1view(kt, hh):
                blk = 128 * 2 * HE
                base = (kt * 2 + hh) * blk
                return w1t_d[base:base + blk].rearrange(
                    "(p j e) -> p j e", p=128, j=2)

            wc_sb = pw.tile([128, E_TILES, C], bf16, tag="wc", name="wc")

            feats = [pw.tile([128, S_LOC], f32, tag=f"fs_{e}", name=f"fs_{e}")
                     for e in range(E_TILES)]

            def lhsT(kt, e):
                if mode == "fp8":
                    hh, e2 = divmod(e, 8)
                    return w1_sb[kt][hh][:, :, e2 * 128:(e2 + 1) * 128]
                return w1_sb[kt][:, e * 128:(e + 1) * 128]

            def rhs(xt, kt, w):
                if mode == "fp8":
                    return xt[:, 2 * kt:2 * kt + 2, :w]
                return xt[:, kt, :w]

            featsb = [pw.tile([128, S_LOC], bf16, tag=f"fb_{e}", name=f"fb_{e}")
                      for e in range(E_TILES)]

            def relu_pool(ps, w, e, s0, last=False):
                segs = w // J
                zr = pz.tile([128, CHUNK], f32, tag="zr", name="zr")
                nc.scalar.activation(zr[:, :w], ps[:, :w],
                                     mybir.ActivationFunctionType.Relu)
                nc.vector.tensor_reduce(
                    out=feats[e][:, s0:s0 + segs],
                    in_=zr[:, :w].rearrange("p (s j) -> p s j", j=J),
                    axis=mybir.AxisListType.X,
                    op=mybir.AluOpType.add,
                )
                if last:
                    # feats[e] is complete once the last chunk's pool ran;
                    # convert for MM2 right away so MM2 never waits.
                    nc.vector.tensor_copy(featsb[e], feats[e])

            n_dt = 2 * KT if mode == "fp8" else KT

            xt0 = px.tile([128, n_dt, CHUNK], in_dt, tag="xt", name="xt")
            cv0 = xt_d[0:D * CHUNK].rearrange("(p d t) -> p d t", p=128, d=n_dt)

            # ring bring-up is serialized and costs ~5ns per descriptor of
            # the ring's HEAD DMA, so a 1-descriptor micro-DMA brings each
            # HWDGE ring up cheaply before the real slabs queue.
            for nm, eng in (("sy", nc.sync), ("sc", nc.scalar)):
                scr = pw.tile([1, 128], in_dt, tag=f"scr_{nm}",
                              name=f"scr_{nm}")
                if mode == "fp8":
                    eng.dma_start(out=scr, in_=xt_d[0:128].rearrange(
                        "(p t) -> p t", p=1))
                else:
                    eng.dma_start(out=scr, in_=w1t_d[0:1, 0:128])

            if mode == "fp8":
                # sync: x chunk-0 split so the kt0 rhs lands first; the rest
                # of the chunk follows as one 2880B-descriptor DMA.  The h1
                # W1 slabs (e-wave 1, consumed from ~16us) follow on sync so
                # scalar only issues 4 DMAs and is free for relus by ~10.5us
                # (wave-1 PSUM-bank recycling is relu-paced).
                # Priming balance: the first matmul gates on (x0a | kt0-h0),
                # one ~125KB piece per HWDGE queue, so scalar carries only
                # kt0/kt1 h0 and sync interleaves kt2/kt3 h0 into its x0
                # stream in consumption order.  h1 half-slabs (e8..15, ~7us
                # later) queue on sync BEHIND chunk-1's x.  Emission order
                # defines the dataflow graph, so everything chunk-0 reads is
                # emitted here, before the chunk-0 matmuls.
                nc.scalar.dma_start(out=w1_sb[0][0], in_=w1view(0, 0))
                nc.scalar.dma_start(out=w1_sb[1][0], in_=w1view(1, 0))
                nc.sync.dma_start(out=xt0[:, 0:2, :], in_=cv0[:, 0:2, :])
                nc.sync.dma_start(out=xt0[:, 2:4, :], in_=cv0[:, 2:4, :])
                nc.sync.dma_start(out=w1_sb[2][0], in_=w1view(2, 0))
                nc.sync.dma_start(out=xt0[:, 4:n_dt, :], in_=cv0[:, 4:n_dt, :])
                nc.sync.dma_start(out=w1_sb[3][0], in_=w1view(3, 0))
                xt1 = px.tile([128, n_dt, CHUNK], in_dt, tag="xt", name="xt1")
                cv1 = xt_d[D * CHUNK:D * 2 * CHUNK].rearrange(
                    "(p d t) -> p d t", p=128, d=n_dt)
                nc.sync.dma_start(out=xt1, in_=cv1)
                for k in range(KT):
                    nc.sync.dma_start(out=w1_sb[k][1], in_=w1view(k, 1))
            else:
                nc.gpsimd.dma_start(out=xt0[:, 0:4, :], in_=cv0[:, 0:4, :])
                nc.gpsimd.dma_start(out=xt0[:, 4:8, :], in_=cv0[:, 4:8, :])
                for k in range(KT):
                    wpiece_bf16(nc.sync, k, 1)
                for k in range(KT):
                    wpiece_bf16(nc.scalar, k, 0)
                # bf16 fallback keeps the old single wct DMA on scalar.
                nc.scalar.dma_start(out=wc_sb, in_=wct_v)

            with tc.tile_pool(name="ps1", bufs=8, space="PSUM") as pp1:
                # warmup matmuls: ramp the PE p-state while the first real
                # weights/x stream in.  Results are never read.  Shares the
                # "ps" tag so the pool stays at 8 banks.
                wps = pp1.tile([128, CHUNK], f32, tag="ps", name="warm")
                for _ in range(N_WARM):
                    if mode == "fp8":
                        nc.tensor.matmul(wps[:, :WARM_W], dmy[:, :, :128], dmy,
                                         start=True, stop=True, perf_mode=perf)
                    else:
                        nc.tensor.matmul(wps[:, :WARM_W], dmy[:, :128], dmy,
                                         start=True, stop=True)

                for ci, (t0, w) in enumerate(chunks):
                    if ci == 0:
                        xt = xt0
                    elif ci == 1 and mode == "fp8":
                        xt = xt1
                    else:
                        xt = px.tile([128, n_dt, CHUNK], in_dt, tag="xt",
                                     name="xt")
                        cv = xt_d[D * t0:D * (t0 + w)].rearrange(
                            "(p d t) -> p d t", p=128, d=n_dt)
                        nc.sync.dma_start(out=xt[:, :, :w], in_=cv)
                        if mode == "fp8" and ci == 4:
                            # MM2 weights: one big DMA queued on sync BEHIND
                            # chunk-4's x -- the xt pool-slot gating delays
                            # its transfer past the startup-critical window
                            # without touching scalar (busy with relus).
                            nc.sync.dma_start(out=wc_sb, in_=wct_v)
                    s0 = t0 // J
                    if ci == 0 and mode == "fp8":
                        # wave 0 (e0..5): k-outer across 6 parallel psum
                        # groups -- the first MMs only need the kt0 h0
                        # half-slab + the first x k-slices, and each kt row
                        # (1.2us) covers the next half-slab's delivery.
                        # e7..15 then go k-inner (one psum at a time): bank 0
                        # (warmup tile) is free for e7, and from e8 on the
                        # recycled bank's relu (started back at wave-0 kt3)
                        # is already done -- an 8-wide wave instead stalls
                        # ~2.3us waiting for relu0 on the 9th psum alloc.
                        wave = 7
                        pss = [pp1.tile([128, CHUNK], f32, tag="ps",
                                        name=f"ps0_{i}") for i in range(wave)]
                        for kt in range(KT):
                            for i in range(wave):
                                nc.tensor.matmul(
                                    pss[i][:, :w],
                                    lhsT(kt, i),
                                    rhs(xt, kt, w),
                                    start=(kt == 0),
                                    stop=(kt == KT - 1),
                                    perf_mode=perf,
                                )
                        for i in range(wave):
                            relu_pool(pss[i], w, i, s0)
                        e_range = range(wave, E_TILES)
                    elif ci == 0:
                        # bf16 fallback: original two k-outer waves.
                        e0 = 0
                        for wave in (8, 8):
                            pss = [pp1.tile([128, CHUNK], f32, tag="ps",
                                            name=f"ps0_{e0}_{i}")
                                   for i in range(wave)]
                            for kt in range(KT):
                                for i in range(wave):
                                    nc.tensor.matmul(
                                        pss[i][:, :w],
                                        lhsT(kt, e0 + i),
                                        rhs(xt, kt, w),
                                        start=(kt == 0),
                                        stop=(kt == KT - 1),
                                        perf_mode=perf,
                                    )
                            for i in range(wave):
                                relu_pool(pss[i], w, e0 + i, s0)
                            e0 += wave
                        continue
                    else:
                        e_range = range(E_TILES)
                    for e in e_range:
                        ps = pp1.tile([128, CHUNK], f32, tag="ps", name="ps")
                        for kt in range(KT):
                            nc.tensor.matmul(
                                ps[:, :w],
                                lhsT(kt, e),
                                rhs(xt, kt, w),
                                start=(kt == 0),
                                stop=(kt == KT - 1),
                                perf_mode=perf,
                            )
                        relu_pool(ps, w, e, s0, last=(ci == len(chunks) - 1))

                # MM2 + writeback: all output DMAs ride the sync HWDGE ring
                # (idle by now).  The very last [125,500] copy+DMA is split
                # 375/125 so the final completion semaphore fires sooner.
                for sb in range(S_LOC // S_BLK):
                    ob = pw.tile([S_BLK, C], bf16, tag=f"ob_{sb}", name=f"ob_{sb}")
                    rows = slice(sb * S_BLK, (sb + 1) * S_BLK)
                    # last s-block tapers its c-chunks (LDWEIGHTS pipelines
                    # >=2 deep, so 250-row matmuls still stream at full
                    # rate): the final writeback piece is half the bytes,
                    # and ~45 B/ns per ring makes the tail byte-bound.
                    cchunks = ((0, C_BLK), (C_BLK, C_BLK)) \
                        if sb < S_LOC // S_BLK - 1 else \
                        ((0, C_BLK), (C_BLK, 250), (C_BLK + 250, 250))
                    for c0, cw in cchunks:
                        ps = pp1.tile([S_BLK, C_BLK], f32, tag="ps", name="ps2")
                        for e in range(E_TILES):
                            nc.tensor.matmul(
                                ps[:, :cw],
                                featsb[e][:, rows],
                                wc_sb[:, e, c0:c0 + cw],
                                start=(e == 0),
                                stop=(e == E_TILES - 1),
                            )
                        # writeback split by PARTITIONS across the two HWDGE
                        # rings: a ring moves ~45 B/ns here (2 DMA engines x
                        # 22.5 B/ns), so one 125KB piece holds its queue for
                        # ~2.8us and head-of-line-blocks the final piece.
                        # 63+62 rows per ring keeps both queues drained and
                        # the last completion semaphore fires ~1.5us sooner.
                        cs = slice(c0, c0 + cw)
                        r0 = sb * S_BLK
                        nc.scalar.copy(ob[:, cs], ps[:, :cw])
                        if (sb, c0) == (S_LOC // S_BLK - 1, C_BLK):
                            # second-to-last piece rides the otherwise-idle
                            # gpsimd SWDGE queue (its ~3-4us latency hides
                            # under the final block's matmuls) so both
                            # HWDGE rings are EMPTY when the final piece's
                            # descriptors arrive -- the drain is bound at
                            # ~45ns per descriptor per queue, so HoL
                            # blocking there directly delays the barrier.
                            nc.gpsimd.dma_start(out=out_d[rows, cs],
                                                in_=ob[:, cs])
                        else:
                            nc.sync.dma_start(
                                out=out_d[r0:r0 + 63, cs], in_=ob[0:63, cs])
                            nc.scalar.dma_start(
                                out=out_d[r0 + 63:r0 + S_BLK, cs],
                                in_=ob[63:S_BLK, cs])

    nc.compile()
    return nc


def _prep_inputs(x, W1, Wlin, mode=MODE):
    wc = (Wlin[:, :E] + Wlin[:, E:]) / np.float32(J)     # [C, E] f32
    if mode == "fp8":
        in_np = FP8
        W1 = W1 * np.float32(W1_SCALE)
        wc = wc / np.float32(W1_SCALE)
    else:
        in_np = BF16
    wct = np.ascontiguousarray(wc.T).astype(BF16)        # [E, C] bf16
    if mode == "fp8":
        # prepack to [kt][half][p][j][e']: row kt*256 + j*128 + p,
        # col half*1024 + e'  (one 2KB descriptor per partition per slab)
        KT = 4
        a = W1.T.reshape(KT, 2, 128, 2, HE)              # [kt][j][p][h][e']
        w1t = np.ascontiguousarray(
            a.transpose(0, 3, 2, 1, 4)).astype(in_np).ravel()
    else:
        w1t = np.ascontiguousarray(W1.T).astype(in_np)   # [D, E]
    in_maps = []
    for c in range(N_CORES):
        xs = x[c * T_LOC:(c + 1) * T_LOC]                # [7500, 1024]
        pieces = []
        t0 = 0
        while t0 < T_LOC:                                # [p][d_tile][t] chunks
            w = min(CHUNK, T_LOC - t0)
            blk = xs[t0:t0 + w].T.reshape(8, 128, w).transpose(1, 0, 2)
            pieces.append(np.ascontiguousarray(blk).astype(in_np).ravel())
            t0 += w
        xt = np.concatenate(pieces)                      # [D*T_LOC] flat
        in_maps.append({"xt": xt, "w1t": w1t, "wct": wct})
    return in_maps


def _run(in_maps, mode=MODE, trace=False, **kw):
    from concourse.bass_utils import run_bass_kernel_spmd

    if mode not in _cache:
        _cache[mode] = _build(mode)
    res = run_bass_kernel_spmd(_cache[mode], in_maps,
                               core_ids=list(range(N_CORES)), trace=trace, **kw)
    logits = np.concatenate(
        [np.asarray(r["out"]) for r in res.results], axis=0).astype(np.float32)
    return logits, res


def kernel(x, y, W1, W2, Wlin):
    x = np.asarray(x, dtype=np.float32)
    W1 = np.asarray(W1, dtype=np.float32)
    Wlin = np.asarray(Wlin, dtype=np.float32)
    modes = (MODE, "bf16") if MODE != "bf16" else ("bf16",)
    for i, mode in enumerate(modes):
        try:
            logits, _ = _run(_prep_inputs(x, W1, Wlin, mode=mode), mode=mode)
            return logits
        except Exception:
            if i == len(modes) - 1:
                raise
    raise RuntimeError("unreachable")
